# revision 50
# baseline (speedup 1.0000x reference)
"""Trainium2 Bass kernel for nn_CAFF_3100966388292 (fp8, wide-PSUM pipelined).

Dual-stream (SAR/OPT) cross-attention fusion net:
  theta/phi/g 1x1-conv projections on both streams, per-sample NxN attention
  maps fused elementwise, both value streams attended, product taken, output
  1x1-conv + residual + channel-mean pool + linear head.
Pure data parallel over batch: 4 samples per core on 8 cores.

Changes over the split-PSUM baseline (~112us):
  * PSUM re-pooled as ONE ring of 4 x [128,768] tiles (2 banks each, all 8
    banks): every exp / cast / copy / mul drain is ONE 768-wide instruction
    instead of a 512+256 pair (the ACT engine has exec-queue depth 0, so
    each instruction pays its full ~160-200ns fixed cost), while the
    depth-4 rotation absorbs the ~2.7us exp backlog the Scalar queue
    carries across each section boundary - depth 3 stalled the next
    section's first psum allocations behind it.
  * g-projection psums packed three mc-chunks per wide tile (one cast per
    three chunks); Z rows, the zcol/qcol columns and the head psum share
    the same ring.
  * The deferred attention-apply is split across the next sample's sections
    (cic0 into the x-section, cic1 into the y-section) instead of both in
    the y-section: the y-section is Scalar-bound (6 exps + casts) while the
    x-section has Scalar slack.
  * Sample s+1's input DMAs issue at the TOP of sample s's body (issued at
    the end they land ~1.7us into s+1's x-section and stall its theta
    projection every sample).
  * The ACT activation-table load (1.28us) no longer fires mid-kernel: the
    warm-up runs on a memset tile with no DMA dependency, so the hoisted
    table load executes during the input-DMA window.  exp's logit shift
    bias, the transpose identity and the Z ones-column are memset on-device
    (expb/ident/ones2 DMAs dropped), and the theta/phi bias path is
    specialized out when the biases are zero (tb DMA dropped) - the sync
    DMA queue issues weights only, so the first matmul's dependencies land
    sooner.
  * The PE's DVFS ramp (full clock only after ~3us of continuous execution;
    216ns vs 330-590ns per 512-col fp8-DR matmul) is burned on dummy
    matmuls over memset data inside the head's input-DMA window, so the
    first real projections run at full speed.
  * Latency-trimmed last-sample tail: the final attention-map pair is
    multiplied per-mc, the sample's own Zx ones-matmuls fill its y-loop's
    exp-paced stall window, and the Ux bounce runs on Scalar (the tail's
    DVE queue holds the p1 product and yv muls), shortening the serial
    qraw chain.
"""

import sys
import types

import ml_dtypes
import numpy as np

# The agent image's antenv package lacks axon_hooks; register the equivalent
# NTFF hook so run_bass_kernel_spmd(trace=True) works if ever requested.
try:  # pragma: no cover
    import antenv.axon_hooks  # noqa: F401
except ImportError:
    try:
        from trn_agent_boot.trn_boot import _ntff_profile_via_ctypes

        _hook = _ntff_profile_via_ctypes("/opt/axon/libaxon_pjrt.so")
        _mod = types.ModuleType("antenv.axon_hooks")
        _mod.get_axon_ntff_profile_hook = lambda: _hook
        _mod.set_axon_ntff_profile_hook = lambda h: None
        sys.modules["antenv.axon_hooks"] = _mod
    except Exception:
        pass

import concourse.bass as bass
import concourse.tile as tile
from concourse import bacc, mybir
from concourse.bass_utils import run_bass_kernel_spmd

F32 = mybir.dt.float32
BF16 = mybir.dt.bfloat16
FP8 = mybir.dt.float8e4
FP8W = mybir.dt.float8e5  # wide-range fp8 for exp maps
EXP_SHIFT = -12.0  # constant logit shift before exp; cancels exactly in the math

B, C, CI, N, HOUT = 32, 512, 256, 768, 256
NCORES = 8
BPC = B // NCORES  # samples per core
KC = C // 128  # 4 k-chunks over channels
MC = N // 128  # 6 chunks over positions
CIC = CI // 128  # 2 chunks over inner channels
# free-dim split of N into PSUM-bank-legal matmul halves
NH = ((0, 512), (512, 256))

_cached = {}


def _pack(a):
    """(R, F) host array -> (128, R//128 * F) partition-major fp8e4."""
    a = np.asarray(a, dtype=np.float32)
    r, f = a.shape
    k = r // 128
    return np.ascontiguousarray(
        a.reshape(k, 128, f).transpose(1, 0, 2).reshape(128, k * f)
    ).astype(ml_dtypes.float8_e4m3fn)


def _build(has_tb, has_gb_x, has_gb_y, has_hb):
    nc = bacc.Bacc("TRN2", target_bir_lowering=False, debug=False)
    AF = mybir.ActivationFunctionType

    def mm(out, lhsT, rhs, start, stop, **kw):
        nc.tensor.matmul(out, lhsT, rhs, start=start, stop=stop, **kw)

    def mmdr(out, lhsT, rhs, start, stop, **kw):
        nc.tensor.matmul(out, lhsT, rhs, start=start, stop=stop,
                         perf_mode=mybir.MatmulPerfMode.DoubleRow, **kw)

    # inputs host-packed to (BPC, 128, KC*N) partition-major fp8e4
    d_x8 = nc.dram_tensor("sar8", [BPC, 128, KC * N], FP8, kind="ExternalInput")
    d_y8 = nc.dram_tensor("opt8", [BPC, 128, KC * N], FP8, kind="ExternalInput")
    # host-pretransposed + packed projection weights, (128, KC*CI) fp8e4
    d_w = {
        nm: nc.dram_tensor(nm, [128, KC * CI], FP8, kind="ExternalInput")
        for nm in ("wt_tx", "wt_px", "wt_ty", "wt_py", "wt_gx", "wt_gy")
    }
    d_hwT = nc.dram_tensor("hwT", [128, MC * HOUT], BF16, kind="ExternalInput")
    d_wbar = nc.dram_tensor("wbar", [CI], BF16, kind="ExternalInput")
    d_rs = nc.dram_tensor("rs", [BPC, 128, MC], F32, kind="ExternalInput")
    if has_tb:
        # theta/phi bias columns batched into one DMA: rows = (tx, px, ty, py)
        d_tb = nc.dram_tensor("tb", [4, CI], F32, kind="ExternalInput")
    need_onesr = has_gb_x or has_gb_y or has_hb
    if need_onesr:
        d_onesr = nc.dram_tensor("ones_row", [1, 128], BF16, kind="ExternalInput")
    d_gb = {}
    if has_gb_x:
        d_gb["x"] = nc.dram_tensor("gb_x", [1, CI], BF16, kind="ExternalInput")
    if has_gb_y:
        d_gb["y"] = nc.dram_tensor("gb_y", [1, CI], BF16, kind="ExternalInput")
    if has_hb:
        d_hb = nc.dram_tensor("hb", [1, HOUT], BF16, kind="ExternalInput")
    d_out = nc.dram_tensor("out", [BPC, HOUT], F32, kind="ExternalOutput")

    with tile.TileContext(nc) as tc, \
            tc.tile_pool(name="wts", bufs=1) as wts, \
            tc.tile_pool(name="inp", bufs=2) as inp, \
            tc.tile_pool(name="proj", bufs=2) as proj, \
            tc.tile_pool(name="att", bufs=2) as attp, \
            tc.tile_pool(name="yvp", bufs=2) as yvp, \
            tc.tile_pool(name="rows", bufs=1) as rows, \
            tc.tile_pool(name="rtmp", bufs=2) as rtmp, \
            tc.tile_pool(name="psM", bufs=4, space="PSUM") as psM:

        # ---- DMAs in strict first-use order: the queues are FIFO, so
        # everything emitted ahead of the first matmul's dependencies delays
        # kernel start.  The sync queue now carries weights only. ----
        def load_w(nm):
            t = wts.tile([128, KC, CI], FP8, tag=nm, name=nm)
            nc.sync.dma_start(t[:], d_w[nm].ap().rearrange("p (k f) -> p k f", k=KC))
            return t

        # inputs issue their descriptors from the otherwise-idle GpSimd
        # sequencer so they don't serialize behind the weight DMAs on Sync
        w_sb = {"wt_tx": load_w("wt_tx")}
        x8_0 = inp.tile([128, KC, N], FP8, tag="x8", name="x8")
        nc.gpsimd.dma_start(x8_0[:, 0:2, :],
                            d_x8[0][:, :2 * N].rearrange("p (k n) -> p k n", k=2))
        w_sb["wt_px"] = load_w("wt_px")
        nc.gpsimd.dma_start(x8_0[:, 2:, :],
                            d_x8[0][:, 2 * N:].rearrange("p (k n) -> p k n", k=2))
        y8_0 = inp.tile([128, KC, N], FP8, tag="y8", name="y8")
        nc.gpsimd.dma_start(y8_0[:], d_y8[0].rearrange("p (k n) -> p k n", k=KC))
        # constants built on-device (no DMA): transpose identity + the Z ones
        # column (dual-row ldweights needs a 16B-aligned even stride between
        # the two k-rows of lhsT, so the ones column is padded to [128,2,16])
        ident = wts.tile([1, 1], F32, tag="ident", name="ident")
        nc.vector.memset(ident[:], 1.0)
        ones2 = wts.tile([128, 2, 16], FP8W, tag="ones2", name="ones2")
        nc.vector.memset(ones2[:], 1.0)
        expb = wts.tile([128, 1], F32, tag="expb", name="expb")
        nc.vector.memset(expb[:], EXP_SHIFT)
        # burn the PE's DVFS ramp on dummy matmuls inside the input-DMA
        # window: the Tensor engine reaches full clock only after ~3us of
        # continuous execution, so without this the first ~12 real matmuls
        # run at half speed (330-590ns instead of 216ns per 512 columns)
        dummy = wts.tile([128, 2, 512], FP8, tag="dummy", name="dummy")
        nc.vector.memset(dummy[:], 1.0)
        dps = psM.tile([128, 512], F32, tag="M", name="dps")
        for _ in range(7):
            mmdr(dps[0:16, :], dummy[:, :, 0:16], dummy[:], True, True)
        dscrap = wts.tile([16, 512], FP8, tag="dscrap", name="dscrap")
        nc.vector.tensor_copy(dscrap[:], dps[0:16, :])
        # pre-warm the Scalar activation table while the engine is idle: the
        # lazy ACT_TABLE_LOAD (1.3us) otherwise fires on the first theta
        # cast, inside sample 0's critical chain.  The warm-up depends only
        # on the memset tile, so it runs during the input-DMA window.
        warm = rtmp.tile([1, 1], F32, tag="warm", name="warm")
        nc.scalar.activation(warm[:], ident[:], AF.Identity)
        nc.scalar.activation(warm[:], ident[:], AF.Exp)
        nc.scalar.activation(warm[:], ident[:], AF.Square)
        w_sb["wt_gx"] = load_w("wt_gx")
        w_sb["wt_ty"] = load_w("wt_ty")
        w_sb["wt_py"] = load_w("wt_py")
        w_sb["wt_gy"] = load_w("wt_gy")
        rs_0 = inp.tile([128, MC], F32, tag="rs", name="rs")
        nc.gpsimd.dma_start(rs_0[:], d_rs[0])

        # ---- small constants (all needed later than the projections) ----
        wbar = wts.tile([128, CIC], BF16, tag="wbar", name="wbar")
        nc.sync.dma_start(wbar[:], d_wbar.ap().rearrange("(k p) -> p k", p=128))
        hwT = wts.tile([128, MC, HOUT], BF16, tag="hwT", name="hwT")
        nc.sync.dma_start(hwT[:], d_hwT.ap().rearrange("p (k f) -> p k f", k=MC))
        tb_sb = {}
        if has_tb:
            tb_all = wts.tile([128, 4, CIC], F32, tag="tb", name="tb_all")
            nc.sync.dma_start(tb_all[:],
                              d_tb.ap().rearrange("s (k p) -> p s k", p=128))
            tb_sb = {nm: tb_all[:, i] for i, nm in
                     enumerate(("b_tx", "b_px", "b_ty", "b_py"))}
        if need_onesr:
            ones_row = wts.tile([1, 128], BF16, tag="ones_row", name="ones_row")
            nc.sync.dma_start(ones_row[:], d_onesr.ap())
        gb_sb = {}
        for st, d in d_gb.items():
            t = wts.tile([1, CI], BF16, tag=f"gb_{st}", name=f"gb_{st}")
            nc.sync.dma_start(t[:], d.ap())
            gb_sb[st] = t
        if has_hb:
            hb = wts.tile([1, HOUT], BF16, tag="hb", name="hb")
            nc.sync.dma_start(hb[:], d_hb.ap())

        def load_inputs(s):
            x8 = inp.tile([128, KC, N], FP8, tag="x8", name="x8")
            y8 = inp.tile([128, KC, N], FP8, tag="y8", name="y8")
            rs_sb = inp.tile([128, MC], F32, tag="rs", name="rs")
            nc.gpsimd.dma_start(x8[:], d_x8[s].rearrange("p (k n) -> p k n", k=KC))
            nc.gpsimd.dma_start(y8[:], d_y8[s].rearrange("p (k n) -> p k n", k=KC))
            nc.gpsimd.dma_start(rs_sb[:], d_rs[s])
            return x8, y8, rs_sb

        in_tiles = [(x8_0, y8_0, rs_0)]

        pooledT = rows.tile([128, MC, BPC], BF16, tag="pooledT", name="pooledT")

        def proj_tile(w, src, bias, dst, on_scalar):
            """theta/phi projection: one [128,768] psum per cic, wide cast."""
            for cic in range(CIC):
                pt = psM.tile([128, N], F32, tag="M", name="projp")
                for kp in range(KC // 2):
                    for o, f in NH:
                        mmdr(pt[:, o:o + f],
                             w[:, 2 * kp:2 * kp + 2, cic * 128:(cic + 1) * 128],
                             src[:, 2 * kp:2 * kp + 2, o:o + f],
                             kp == 0, kp == KC // 2 - 1)
                b = bias[:, cic:cic + 1] if bias is not None else None
                if on_scalar:  # theta casts on Scalar (ACT bias port)
                    nc.scalar.activation(dst[:, cic, :], pt[:], AF.Identity,
                                         bias=b if b is not None else 0.0)
                else:  # phi casts on DVE to balance engine load
                    if b is not None:
                        nc.vector.tensor_scalar_add(dst[:, cic, :], pt[:], b)
                    else:
                        nc.vector.tensor_copy(dst[:, cic, :], pt[:])

        def emit_Z_key(E_st):
            """one softmax denominator row via fp8-DR ones-matmuls into a
            [1,768] psum row (both NH halves of one wide tile)."""
            zt = psM.tile([1, N], F32, tag="M", name="zrow")
            for o, f in NH:
                for jp in range(MC // 2):
                    mmdr(zt[:, o:o + f], ones2[:, :, :1],
                         E_st[:, 2 * jp:2 * jp + 2, o:o + f],
                         jp == 0, jp == MC // 2 - 1)
            return zt

        def emit_Z(fx, zx_sb=None):
            """both denominators, one wide copy + one wide row-multiply."""
            s, E, S, gT, rs_sb = fx
            if zx_sb is None:
                zx = emit_Z_key(E["x"])
                zx_sb = rtmp.tile([1, N], F32, tag="zx_sb", name="zx_sb")
                # Scalar (which has slack here) frees the zx psum fast
                nc.scalar.copy(zx_sb[:], zx[:])
            zy = emit_Z_key(E["y"])
            p1 = rtmp.tile([1, N], F32, tag="p1", name="p1")
            nc.vector.tensor_mul(p1[:], zx_sb[:], zy[:])
            return p1

        def emit_T(p1):
            """Zx*Zy row -> columns; R2col = 1/(ZxZy)^2 as tiny column ops."""
            zcol = psM.tile([128, MC], F32, tag="M", name="zcol")
            for j in range(MC):
                nc.tensor.transpose(zcol[:, j:j + 1],
                                    p1[:, j * 128:(j + 1) * 128], ident[:])
            sq = rtmp.tile([128, MC], F32, tag="sq", name="sq")
            nc.scalar.activation(sq[:], zcol[:], AF.Square)
            rcol = rtmp.tile([128, MC], F32, tag="rcol", name="rcol")
            nc.vector.reciprocal_approx_fast(rcol[:], sq[:])
            return rcol

        def emit_U_cic(fx, yv, cic, copy_on_scalar):
            """unnormalized attention-apply (fp8-DR) + product, one cic."""
            s, E, S, gT, rs_sb = fx
            ptu = {}
            for st in ("x", "y"):
                ptu[st] = psM.tile([128, N], F32, tag="M", name=f"U{st}")
                for o, f in NH:
                    for jp in range(MC // 2):
                        mmdr(ptu[st][:, o:o + f],
                             gT[st][:, 2 * jp:2 * jp + 2,
                                    cic * 128:(cic + 1) * 128],
                             S[:, 2 * jp:2 * jp + 2, o:o + f],
                             jp == 0, jp == MC // 2 - 1)
            # DVE tensor_tensor cannot read two PSUM operands; bounce Ux
            # through SBUF.  The bounce engine is picked per section: Scalar
            # in the exp-light section, DVE where Scalar is exp-bound.
            ux_sb = yvp.tile([128, N], BF16, tag="ux_sb", name="ux_sb")
            if copy_on_scalar:
                nc.scalar.copy(ux_sb[:], ptu["x"][:])
            else:
                nc.vector.tensor_copy(ux_sb[:], ptu["x"][:])
            nc.vector.tensor_mul(yv[:, cic, :], ux_sb[:], ptu["y"][:])

        def emit_Q(fx, yv, rcol):
            """qraw directly in column form + pooled fixup into pooledT."""
            s, E, S, gT, rs_sb = fx
            qcol = psM.tile([128, MC], F32, tag="M", name="qcol")
            for j in range(MC):
                for cic in range(CIC):
                    mm(qcol[:, j:j + 1], yv[:, cic, j * 128:(j + 1) * 128],
                       wbar[:, cic:cic + 1], cic == 0, cic == CIC - 1)
            pm = rtmp.tile([128, MC], F32, tag="pm", name="pm")
            nc.vector.tensor_mul(pm[:], rcol[:], qcol[:])
            nc.vector.tensor_add(pooledT[:, :, s], pm[:], rs_sb[:])

        # Software pipeline: sample s's exp-dependent stages (Z, U, fixup)
        # are deferred into sample s+1's sections, where every exp of sample
        # s has long finished - the PE never waits on Scalar.  The
        # attention-apply is split: cic0 into the x-section (which has
        # Scalar slack), cic1 into the exp-heavy y-section.
        prev = None
        prev_yv = None
        prev_p1 = None
        prev_rcol = None
        for s in range(BPC):
            x8, y8, rs_sb = in_tiles[s]
            # prefetch sample s+1's inputs a full section ahead: issued at
            # the end of the body they land ~1.7us into s+1's x-section and
            # stall its theta projection every sample
            if s + 1 < BPC:
                in_tiles.append(load_inputs(s + 1))
            streams = (("x", x8), ("y", y8))
            pj = {}
            gT = {}
            E = {}
            S = attp.tile([128, MC, N], FP8W, tag="S", name="S")
            for st, src in streams:
                for pr in ("t", "p"):
                    dst = proj.tile([128, CIC, N], FP8, tag=f"pj_{pr}{st}",
                                    name=f"pj_{pr}{st}")
                    pj[pr + st] = dst
                    proj_tile(w_sb[f"wt_{pr}{st}"], src,
                              tb_sb.get(f"b_{pr}{st}"), dst, pr == "t")
                # deferred stages of the previous sample
                if prev is not None:
                    if st == "x":
                        prev_p1 = emit_Z(prev)
                        prev_yv = yvp.tile([128, CIC, N], BF16, tag="yv",
                                           name="yv")
                        emit_U_cic(prev, prev_yv, 0, False)
                    else:
                        emit_U_cic(prev, prev_yv, 1, True)

                # logits interleaved 1:1 with g tiles: the Scalar EXP stream
                # trails the logits tiles; the g tiles in between drain via
                # DVE, so the psum rotation never stalls the PE on a
                # pending exp
                wg = w_sb[f"wt_g{st}"]
                gdst = proj.tile([128, MC, CI], FP8, tag=f"gT{st}",
                                 name=f"gT{st}")
                gT[st] = gdst
                has_b = st in gb_sb
                edst = attp.tile([128, MC, N], FP8W, tag=f"E{st}", name=f"E{st}")
                E[st] = edst
                gps = None
                for mc_ in range(MC):
                    # g psums packed three mc-chunks per wide tile
                    if mc_ % 3 == 0:
                        gps = psM.tile([128, 3, CI], F32, tag="M", name="gps")
                    gh = gps[:, mc_ % 3]
                    for kp in range(KC // 2):
                        mmdr(gh,
                             src[:, 2 * kp:2 * kp + 2, mc_ * 128:(mc_ + 1) * 128],
                             wg[:, 2 * kp:2 * kp + 2, :],
                             kp == 0, (kp == KC // 2 - 1) and not has_b)
                    if has_b:
                        mm(gh, ones_row[:], gb_sb[st][:], False, True,
                           skip_group_check=True)
                    if mc_ % 3 == 2:
                        nc.vector.tensor_copy(gdst[:, mc_ - 2:mc_ + 1, :],
                                              gps[:])
                    lt = psM.tile([128, N], F32, tag="M", name="logits")
                    for o, f in NH:
                        mmdr(lt[:, o:o + f],
                             pj["p" + st][:, :, mc_ * 128:(mc_ + 1) * 128],
                             pj["t" + st][:, :, o:o + f], True, True)
                    nc.scalar.activation(edst[:, mc_, :], lt[:], AF.Exp,
                                         bias=expb[:])
                    if st == "y" and s == BPC - 1 and mc_ >= MC - 2:
                        # last sample: final pair split per-mc so the tail's
                        # attention-apply chain starts half a mul earlier
                        nc.vector.tensor_mul(S[:, mc_:mc_ + 1, :],
                                             E["x"][:, mc_:mc_ + 1, :],
                                             E["y"][:, mc_:mc_ + 1, :])
                        if mc_ == MC - 2:
                            # its own Zx matmuls fill the y-loop's exp-paced
                            # stall window (E-x complete since the x-section)
                            # and the zx drain runs here, between exps, so
                            # the tail's psum ring never waits on it behind
                            # the trailing exp backlog
                            last_zx = emit_Z_key(E["x"])
                            last_zx_sb = rtmp.tile([1, N], F32, tag="zx_sb",
                                                   name="zx_sb")
                            nc.scalar.copy(last_zx_sb[:], last_zx[:])
                    elif st == "y" and mc_ % 2 == 1:
                        # fused map product per chunk-pair (adjacent free dim)
                        nc.vector.tensor_mul(S[:, mc_ - 1:mc_ + 1, :],
                                             E["x"][:, mc_ - 1:mc_ + 1, :],
                                             E["y"][:, mc_ - 1:mc_ + 1, :])
                    elif st == "x" and mc_ == MC - 1 and prev is not None:
                        prev_rcol = emit_T(prev_p1)
                if st == "y" and prev is not None:
                    emit_Q(prev, prev_yv, prev_rcol)

            prev = (s, E, S, gT, rs_sb)

        # drain the last sample
        p1 = emit_Z(prev, last_zx_sb)
        yv = yvp.tile([128, CIC, N], BF16, tag="yv", name="yv")
        # tail: with the zx drain moved into the y-loop, the Scalar queue
        # still holds the trailing exps, so bounce Ux through DVE
        emit_U_cic(prev, yv, 0, False)
        rcol = emit_T(p1)
        # cic1 attention-apply with a latency-trimmed tail: the qraw-cic0
        # matvecs run while cic1 is still in flight, and qraw-cic1 follows
        # the ux/yv bounce as it lands
        _, _, S_l, gT_l, rs_l = prev
        ptu = {}
        for st in ("x", "y"):
            ptu[st] = psM.tile([128, N], F32, tag="M", name=f"U{st}l")
            for o, f in NH:
                for jp in range(MC // 2):
                    mmdr(ptu[st][:, o:o + f],
                         gT_l[st][:, 2 * jp:2 * jp + 2, 128:256],
                         S_l[:, 2 * jp:2 * jp + 2, o:o + f],
                         jp == 0, jp == MC // 2 - 1)
        qcol = psM.tile([128, MC], F32, tag="M", name="qcol")
        for j in range(MC):
            mm(qcol[:, j:j + 1], yv[:, 0, j * 128:(j + 1) * 128],
               wbar[:, 0:1], True, False)
        ux_sb = yvp.tile([128, N], BF16, tag="ux_sb", name="ux_sb")
        nc.scalar.copy(ux_sb[:], ptu["x"][:])
        nc.vector.tensor_mul(yv[:, 1, :], ux_sb[:], ptu["y"][:])
        for j in range(MC):
            mm(qcol[:, j:j + 1], yv[:, 1, j * 128:(j + 1) * 128],
               wbar[:, 1:2], False, j == MC - 1)
        pm = rtmp.tile([128, MC], F32, tag="pm", name="pm")
        nc.vector.tensor_mul(pm[:], rcol[:], qcol[:])
        nc.vector.tensor_add(pooledT[:, :, BPC - 1], pm[:], rs_l[:])

        # ---- head ----
        pt = psM.tile([BPC, HOUT], F32, tag="M", name="head_ps")
        for j in range(MC):
            mm(pt[:], pooledT[:, j, :], hwT[:, j, :],
               j == 0, (j == MC - 1) and not has_hb)
        if has_hb:
            mm(pt[:], ones_row[:, :BPC], hb[:], False, True)
        out_sb = rows.tile([BPC, HOUT], F32, tag="out_sb", name="out_sb")
        nc.scalar.copy(out_sb[:], pt[:])
        nc.sync.dma_start(d_out[:], out_sb[:])

    nc.compile()
    return nc


def _prepare(inputs):
    f = lambda k: np.ascontiguousarray(np.asarray(inputs[k], dtype=np.float32))
    bf = lambda a: np.ascontiguousarray(np.asarray(a, dtype=ml_dtypes.bfloat16))
    sar, opt = f("sar"), f("opt")
    ga = float(np.asarray(inputs["gamma_att"]).reshape(-1)[0])
    go = float(np.asarray(inputs["gamma_opt"]).reshape(-1)[0])
    gs = float(np.asarray(inputs["gamma_sar"]).reshape(-1)[0])
    W_w, W_b = f("W_w"), f("W_b")
    head_w, head_b = f("head_w"), f("head_b")

    wbar = (ga / C) * W_w.sum(axis=0)  # (CI,)
    bbar = (ga / C) * float(W_b.sum())
    # fold the pooled-constant through the head: out += bbar * head_w.sum(1)
    hb_eff = head_b + bbar * head_w.sum(axis=1)  # (HOUT,)

    tb = np.stack([f("theta_sar_b"), f("phi_sar_b"),
                   f("theta_opt_b"), f("phi_opt_b")])
    has_tb = bool(np.any(tb))
    gb_x, gb_y = f("g_sar_b"), f("g_opt_b")
    has_gb_x = bool(np.any(gb_x))
    has_gb_y = bool(np.any(gb_y))
    has_hb = bool(np.any(hb_eff))

    key = (has_tb, has_gb_x, has_gb_y, has_hb)
    if key not in _cached:
        _cached[key] = _build(*key)
    nc = _cached[key]

    # pack inputs: (B, C, N) -> per-core (BPC, 128, KC*N) partition-major fp8
    def pack_in(a):
        a = a.reshape(B, KC, 128, N).transpose(0, 2, 1, 3).reshape(B, 128, KC * N)
        return np.ascontiguousarray(a).astype(ml_dtypes.float8_e4m3fn)

    sar_p, opt_p = pack_in(sar), pack_in(opt)

    # exact residual + channel-mean pool term, per-sample column layout
    rs = (go / C) * opt.sum(axis=1) + (gs / C) * sar.sum(axis=1)  # (B, N)
    rs = np.ascontiguousarray(
        rs.reshape(B, MC, 128).transpose(0, 2, 1)).astype(np.float32)

    common = {
        "wt_tx": _pack(f("theta_sar_w").T),
        "wt_px": _pack(f("phi_sar_w").T),
        "wt_ty": _pack(f("theta_opt_w").T),
        "wt_py": _pack(f("phi_opt_w").T),
        "wt_gx": _pack(f("g_sar_w").T),
        "wt_gy": _pack(f("g_opt_w").T),
        "hwT": np.ascontiguousarray(
            head_w.T.reshape(MC, 128, HOUT).transpose(1, 0, 2)
            .reshape(128, MC * HOUT)).astype(ml_dtypes.bfloat16),
        "wbar": bf(wbar),
    }
    if has_tb:
        common["tb"] = np.ascontiguousarray(tb)
    if has_gb_x or has_gb_y or has_hb:
        common["ones_row"] = np.ones((1, 128), ml_dtypes.bfloat16)
    if has_gb_x:
        common["gb_x"] = bf(gb_x.reshape(1, CI))
    if has_gb_y:
        common["gb_y"] = bf(gb_y.reshape(1, CI))
    if has_hb:
        common["hb"] = bf(hb_eff.reshape(1, HOUT))

    in_maps = []
    for c in range(NCORES):
        m = dict(common)
        m["sar8"] = np.ascontiguousarray(sar_p[c * BPC:(c + 1) * BPC])
        m["opt8"] = np.ascontiguousarray(opt_p[c * BPC:(c + 1) * BPC])
        m["rs"] = np.ascontiguousarray(rs[c * BPC:(c + 1) * BPC])
        in_maps.append(m)
    return nc, in_maps


def kernel(**inputs):
    nc, in_maps = _prepare(inputs)
    res = run_bass_kernel_spmd(nc, in_maps, core_ids=list(range(NCORES)))
    return np.concatenate([res.results[c]["out"] for c in range(NCORES)], axis=0)


if __name__ == "__main__":
    rng = np.random.default_rng(0)
    ins = {
        "sar": rng.standard_normal((B, C, N), dtype=np.float32),
        "opt": rng.standard_normal((B, C, N), dtype=np.float32),
    }
    for nm in ("g_sar", "g_opt", "theta_sar", "theta_opt", "phi_sar", "phi_opt"):
        ins[nm + "_w"] = 0.02 * rng.standard_normal((CI, C), dtype=np.float32)
        ins[nm + "_b"] = np.zeros((CI,), np.float32)
    ins["W_w"] = 0.02 * rng.standard_normal((C, CI), dtype=np.float32)
    ins["W_b"] = np.zeros((C,), np.float32)
    ins["head_w"] = 0.02 * rng.standard_normal((HOUT, N), dtype=np.float32)
    ins["head_b"] = np.zeros((HOUT,), np.float32)
    ins["gamma_sar"] = np.asarray([0.3], np.float32)
    ins["gamma_opt"] = np.asarray([1.0], np.float32)
    ins["gamma_att"] = np.asarray([1.0], np.float32)
    out = kernel(**ins)
    print(out.shape, out.dtype, np.abs(out).mean())


# revision 51
# speedup vs baseline: 1.0334x; 1.0334x over previous
"""Trainium2 Bass kernel for nn_CAFF_3100966388292 (fp8, wide-PSUM pipelined).

Dual-stream (SAR/OPT) cross-attention fusion net:
  theta/phi/g 1x1-conv projections on both streams, per-sample NxN attention
  maps fused elementwise, both value streams attended, product taken, output
  1x1-conv + residual + channel-mean pool + linear head.
Pure data parallel over batch: 4 samples per core on 8 cores.

Changes over the split-PSUM baseline (~112us):
  * PSUM re-pooled as ONE ring of 4 x [128,768] tiles (2 banks each, all 8
    banks): every exp / cast / copy / mul drain is ONE 768-wide instruction
    instead of a 512+256 pair (the ACT engine has exec-queue depth 0, so
    each instruction pays its full ~160-200ns fixed cost), while the
    depth-4 rotation absorbs the ~2.7us exp backlog the Scalar queue
    carries across each section boundary - depth 3 stalled the next
    section's first psum allocations behind it.
  * g-projection psums packed three mc-chunks per wide tile (one cast per
    three chunks); Z rows, the zcol/qcol columns and the head psum share
    the same ring.
  * The deferred attention-apply is split across the next sample's sections
    (cic0 into the x-section, cic1 into the y-section) instead of both in
    the y-section: the y-section is Scalar-bound (6 exps + casts) while the
    x-section has Scalar slack.
  * Sample s+1's input DMAs issue at the TOP of sample s's body (issued at
    the end they land ~1.7us into s+1's x-section and stall its theta
    projection every sample).
  * The ACT activation-table load (1.28us) no longer fires mid-kernel: the
    warm-up runs on a memset tile with no DMA dependency, so the hoisted
    table load executes during the input-DMA window.  exp's logit shift
    bias, the transpose identity and the Z ones-column are memset on-device
    (expb/ident/ones2 DMAs dropped), and the theta/phi bias path is
    specialized out when the biases are zero (tb DMA dropped) - the sync
    DMA queue issues weights only, so the first matmul's dependencies land
    sooner.
  * The PE's DVFS ramp (full clock only after ~3us of continuous execution;
    216ns vs 330-590ns per 512-col fp8-DR matmul) is burned on dummy
    matmuls over memset data inside the head's input-DMA window, so the
    first real projections run at full speed.
  * Latency-trimmed last-sample tail: the final attention-map pair is
    multiplied per-mc, the sample's own Zx ones-matmuls fill its y-loop's
    exp-paced stall window, and the Ux bounce runs on Scalar (the tail's
    DVE queue holds the p1 product and yv muls), shortening the serial
    qraw chain.
"""

import sys
import types

import ml_dtypes
import numpy as np

# The agent image's antenv package lacks axon_hooks; register the equivalent
# NTFF hook so run_bass_kernel_spmd(trace=True) works if ever requested.
try:  # pragma: no cover
    import antenv.axon_hooks  # noqa: F401
except ImportError:
    try:
        from trn_agent_boot.trn_boot import _ntff_profile_via_ctypes

        _hook = _ntff_profile_via_ctypes("/opt/axon/libaxon_pjrt.so")
        _mod = types.ModuleType("antenv.axon_hooks")
        _mod.get_axon_ntff_profile_hook = lambda: _hook
        _mod.set_axon_ntff_profile_hook = lambda h: None
        sys.modules["antenv.axon_hooks"] = _mod
    except Exception:
        pass

import concourse.bass as bass
import concourse.tile as tile
from concourse import bacc, mybir
from concourse.bass_utils import run_bass_kernel_spmd

F32 = mybir.dt.float32
BF16 = mybir.dt.bfloat16
FP8 = mybir.dt.float8e4
FP8W = mybir.dt.float8e5  # wide-range fp8 for exp maps
EXP_SHIFT = -12.0  # constant logit shift before exp; cancels exactly in the math

B, C, CI, N, HOUT = 32, 512, 256, 768, 256
NCORES = 8
BPC = B // NCORES  # samples per core
KC = C // 128  # 4 k-chunks over channels
MC = N // 128  # 6 chunks over positions
CIC = CI // 128  # 2 chunks over inner channels
# free-dim split of N into PSUM-bank-legal matmul halves
NH = ((0, 512), (512, 256))

_cached = {}


def _pack(a):
    """(R, F) host array -> (128, R//128 * F) partition-major fp8e4."""
    a = np.asarray(a, dtype=np.float32)
    r, f = a.shape
    k = r // 128
    return np.ascontiguousarray(
        a.reshape(k, 128, f).transpose(1, 0, 2).reshape(128, k * f)
    ).astype(ml_dtypes.float8_e4m3fn)


def _build(has_tb, has_gb_x, has_gb_y, has_hb):
    nc = bacc.Bacc("TRN2", target_bir_lowering=False, debug=False)
    AF = mybir.ActivationFunctionType

    def mm(out, lhsT, rhs, start, stop, **kw):
        nc.tensor.matmul(out, lhsT, rhs, start=start, stop=stop, **kw)

    def mmdr(out, lhsT, rhs, start, stop, **kw):
        nc.tensor.matmul(out, lhsT, rhs, start=start, stop=stop,
                         perf_mode=mybir.MatmulPerfMode.DoubleRow, **kw)

    # inputs host-packed to (BPC, 128, KC*N) partition-major fp8e4
    d_x8 = nc.dram_tensor("sar8", [BPC, 128, KC * N], FP8, kind="ExternalInput")
    d_y8 = nc.dram_tensor("opt8", [BPC, 128, KC * N], FP8, kind="ExternalInput")
    # host-pretransposed + packed projection weights, (128, KC*CI) fp8e4
    d_w = {
        nm: nc.dram_tensor(nm, [128, KC * CI], FP8, kind="ExternalInput")
        for nm in ("wt_tx", "wt_px", "wt_ty", "wt_py", "wt_gx", "wt_gy")
    }
    d_hwT = nc.dram_tensor("hwT", [128, MC * HOUT], BF16, kind="ExternalInput")
    d_wbar = nc.dram_tensor("wbar", [CI], BF16, kind="ExternalInput")
    d_rs = nc.dram_tensor("rs", [BPC, 128, MC], F32, kind="ExternalInput")
    if has_tb:
        # theta/phi bias columns batched into one DMA: rows = (tx, px, ty, py)
        d_tb = nc.dram_tensor("tb", [4, CI], F32, kind="ExternalInput")
    need_onesr = has_gb_x or has_gb_y or has_hb
    if need_onesr:
        d_onesr = nc.dram_tensor("ones_row", [1, 128], BF16, kind="ExternalInput")
    d_gb = {}
    if has_gb_x:
        d_gb["x"] = nc.dram_tensor("gb_x", [1, CI], BF16, kind="ExternalInput")
    if has_gb_y:
        d_gb["y"] = nc.dram_tensor("gb_y", [1, CI], BF16, kind="ExternalInput")
    if has_hb:
        d_hb = nc.dram_tensor("hb", [1, HOUT], BF16, kind="ExternalInput")
    d_out = nc.dram_tensor("out", [BPC, HOUT], F32, kind="ExternalOutput")

    with tile.TileContext(nc) as tc, \
            tc.tile_pool(name="wts", bufs=1) as wts, \
            tc.tile_pool(name="inp", bufs=2) as inp, \
            tc.tile_pool(name="proj", bufs=2) as proj, \
            tc.tile_pool(name="att", bufs=2) as attp, \
            tc.tile_pool(name="yvp", bufs=2) as yvp, \
            tc.tile_pool(name="rows", bufs=1) as rows, \
            tc.tile_pool(name="rtmp", bufs=2) as rtmp, \
            tc.tile_pool(name="psM", bufs=4, space="PSUM") as psM:

        # ---- DMAs in strict first-use order: the queues are FIFO, so
        # everything emitted ahead of the first matmul's dependencies delays
        # kernel start.  The sync queue now carries weights only. ----
        def load_w(nm):
            t = wts.tile([128, KC, CI], FP8, tag=nm, name=nm)
            nc.sync.dma_start(t[:], d_w[nm].ap().rearrange("p (k f) -> p k f", k=KC))
            return t

        # inputs issue their descriptors from the otherwise-idle GpSimd
        # sequencer so they don't serialize behind the weight DMAs on Sync
        w_sb = {"wt_tx": load_w("wt_tx")}
        x8_0 = inp.tile([128, KC, N], FP8, tag="x8", name="x8")
        nc.gpsimd.dma_start(x8_0[:, 0:2, :],
                            d_x8[0][:, :2 * N].rearrange("p (k n) -> p k n", k=2))
        w_sb["wt_px"] = load_w("wt_px")
        nc.gpsimd.dma_start(x8_0[:, 2:, :],
                            d_x8[0][:, 2 * N:].rearrange("p (k n) -> p k n", k=2))
        y8_0 = inp.tile([128, KC, N], FP8, tag="y8", name="y8")
        nc.gpsimd.dma_start(y8_0[:], d_y8[0].rearrange("p (k n) -> p k n", k=KC))
        # constants built on-device (no DMA): transpose identity + the Z ones
        # column (dual-row ldweights needs a 16B-aligned even stride between
        # the two k-rows of lhsT, so the ones column is padded to [128,2,16])
        ident = wts.tile([1, 1], F32, tag="ident", name="ident")
        nc.vector.memset(ident[:], 1.0)
        ones2 = wts.tile([128, 2, 16], FP8W, tag="ones2", name="ones2")
        nc.vector.memset(ones2[:], 1.0)
        expb = wts.tile([128, 1], F32, tag="expb", name="expb")
        nc.vector.memset(expb[:], EXP_SHIFT)
        # burn the PE's DVFS ramp on dummy matmuls inside the input-DMA
        # window: the Tensor engine reaches full clock only after ~3us of
        # continuous execution, so without this the first ~12 real matmuls
        # run at half speed (330-590ns instead of 216ns per 512 columns)
        dummy = wts.tile([128, 2, 512], FP8, tag="dummy", name="dummy")
        nc.vector.memset(dummy[:], 1.0)
        dps = psM.tile([128, 512], F32, tag="M", name="dps")
        for _ in range(7):
            mmdr(dps[0:16, :], dummy[:, :, 0:16], dummy[:], True, True)
        dscrap = wts.tile([16, 512], FP8, tag="dscrap", name="dscrap")
        nc.vector.tensor_copy(dscrap[:], dps[0:16, :])
        # pre-warm the Scalar activation table while the engine is idle: the
        # lazy ACT_TABLE_LOAD (1.3us) otherwise fires on the first theta
        # cast, inside sample 0's critical chain.  The warm-up depends only
        # on the memset tile, so it runs during the input-DMA window.
        warm = rtmp.tile([1, 1], F32, tag="warm", name="warm")
        nc.scalar.activation(warm[:], ident[:], AF.Identity)
        nc.scalar.activation(warm[:], ident[:], AF.Exp)
        nc.scalar.activation(warm[:], ident[:], AF.Square)
        w_sb["wt_gx"] = load_w("wt_gx")
        w_sb["wt_ty"] = load_w("wt_ty")
        w_sb["wt_py"] = load_w("wt_py")
        w_sb["wt_gy"] = load_w("wt_gy")
        rs_0 = inp.tile([128, MC], F32, tag="rs", name="rs")
        nc.gpsimd.dma_start(rs_0[:], d_rs[0])

        # ---- small constants (all needed later than the projections) ----
        wbar = wts.tile([128, CIC], BF16, tag="wbar", name="wbar")
        nc.sync.dma_start(wbar[:], d_wbar.ap().rearrange("(k p) -> p k", p=128))
        hwT = wts.tile([128, MC, HOUT], BF16, tag="hwT", name="hwT")
        nc.sync.dma_start(hwT[:], d_hwT.ap().rearrange("p (k f) -> p k f", k=MC))
        tb_sb = {}
        if has_tb:
            tb_all = wts.tile([128, 4, CIC], F32, tag="tb", name="tb_all")
            nc.sync.dma_start(tb_all[:],
                              d_tb.ap().rearrange("s (k p) -> p s k", p=128))
            tb_sb = {nm: tb_all[:, i] for i, nm in
                     enumerate(("b_tx", "b_px", "b_ty", "b_py"))}
        if need_onesr:
            ones_row = wts.tile([1, 128], BF16, tag="ones_row", name="ones_row")
            nc.sync.dma_start(ones_row[:], d_onesr.ap())
        gb_sb = {}
        for st, d in d_gb.items():
            t = wts.tile([1, CI], BF16, tag=f"gb_{st}", name=f"gb_{st}")
            nc.sync.dma_start(t[:], d.ap())
            gb_sb[st] = t
        if has_hb:
            hb = wts.tile([1, HOUT], BF16, tag="hb", name="hb")
            nc.sync.dma_start(hb[:], d_hb.ap())

        def load_inputs(s):
            x8 = inp.tile([128, KC, N], FP8, tag="x8", name="x8")
            y8 = inp.tile([128, KC, N], FP8, tag="y8", name="y8")
            rs_sb = inp.tile([128, MC], F32, tag="rs", name="rs")
            nc.gpsimd.dma_start(x8[:], d_x8[s].rearrange("p (k n) -> p k n", k=KC))
            nc.gpsimd.dma_start(y8[:], d_y8[s].rearrange("p (k n) -> p k n", k=KC))
            nc.gpsimd.dma_start(rs_sb[:], d_rs[s])
            return x8, y8, rs_sb

        in_tiles = [(x8_0, y8_0, rs_0)]

        pooledT = rows.tile([128, MC, BPC], BF16, tag="pooledT", name="pooledT")

        def proj_tile(w, src, bias, dst, on_scalar):
            """theta/phi projection: one [128,768] psum per cic, wide cast."""
            for cic in range(CIC):
                pt = psM.tile([128, N], F32, tag="M", name="projp")
                for kp in range(KC // 2):
                    for o, f in NH:
                        mmdr(pt[:, o:o + f],
                             w[:, 2 * kp:2 * kp + 2, cic * 128:(cic + 1) * 128],
                             src[:, 2 * kp:2 * kp + 2, o:o + f],
                             kp == 0, kp == KC // 2 - 1)
                b = bias[:, cic:cic + 1] if bias is not None else None
                if on_scalar:  # theta casts on Scalar (ACT bias port)
                    nc.scalar.activation(dst[:, cic, :], pt[:], AF.Identity,
                                         bias=b if b is not None else 0.0)
                else:  # phi casts on DVE to balance engine load
                    if b is not None:
                        nc.vector.tensor_scalar_add(dst[:, cic, :], pt[:], b)
                    else:
                        nc.vector.tensor_copy(dst[:, cic, :], pt[:])

        def emit_Z_key(E_st):
            """one softmax denominator row via fp8-DR ones-matmuls into a
            [1,768] psum row (both NH halves of one wide tile)."""
            zt = psM.tile([1, N], F32, tag="M", name="zrow")
            for o, f in NH:
                for jp in range(MC // 2):
                    mmdr(zt[:, o:o + f], ones2[:, :, :1],
                         E_st[:, 2 * jp:2 * jp + 2, o:o + f],
                         jp == 0, jp == MC // 2 - 1)
            return zt

        def emit_Z(fx, zx_sb=None):
            """both denominators, one wide copy + one wide row-multiply."""
            s, E, S, gT, rs_sb = fx
            if zx_sb is None:
                zx = emit_Z_key(E["x"])
                zx_sb = rtmp.tile([1, N], F32, tag="zx_sb", name="zx_sb")
                # Scalar (which has slack here) frees the zx psum fast
                nc.scalar.copy(zx_sb[:], zx[:])
            zy = emit_Z_key(E["y"])
            p1 = rtmp.tile([1, N], F32, tag="p1", name="p1")
            nc.vector.tensor_mul(p1[:], zx_sb[:], zy[:])
            return p1

        def emit_T(p1):
            """Zx*Zy row -> columns; R2col = 1/(ZxZy)^2 as tiny column ops."""
            zcol = psM.tile([128, MC], F32, tag="M", name="zcol")
            for j in range(MC):
                nc.tensor.transpose(zcol[:, j:j + 1],
                                    p1[:, j * 128:(j + 1) * 128], ident[:])
            sq = rtmp.tile([128, MC], F32, tag="sq", name="sq")
            nc.scalar.activation(sq[:], zcol[:], AF.Square)
            rcol = rtmp.tile([128, MC], F32, tag="rcol", name="rcol")
            nc.vector.reciprocal_approx_fast(rcol[:], sq[:])
            return rcol

        def emit_U_cic(fx, yv, cic, copy_on_scalar):
            """unnormalized attention-apply (fp8-DR) + product, one cic."""
            s, E, S, gT, rs_sb = fx
            ptu = {}
            for st in ("x", "y"):
                ptu[st] = psM.tile([128, N], F32, tag="M", name=f"U{st}")
                for o, f in NH:
                    for jp in range(MC // 2):
                        mmdr(ptu[st][:, o:o + f],
                             gT[st][:, 2 * jp:2 * jp + 2,
                                    cic * 128:(cic + 1) * 128],
                             S[:, 2 * jp:2 * jp + 2, o:o + f],
                             jp == 0, jp == MC // 2 - 1)
            # DVE tensor_tensor cannot read two PSUM operands; bounce Ux
            # through SBUF.  The bounce engine is picked per section: Scalar
            # in the exp-light section, DVE where Scalar is exp-bound.
            ux_sb = yvp.tile([128, N], BF16, tag="ux_sb", name="ux_sb")
            if copy_on_scalar:
                nc.scalar.copy(ux_sb[:], ptu["x"][:])
            else:
                nc.vector.tensor_copy(ux_sb[:], ptu["x"][:])
            nc.vector.tensor_mul(yv[:, cic, :], ux_sb[:], ptu["y"][:])

        def emit_Q(fx, yv, rcol):
            """qraw directly in column form + pooled fixup into pooledT."""
            s, E, S, gT, rs_sb = fx
            qcol = psM.tile([128, MC], F32, tag="M", name="qcol")
            for j in range(MC):
                for cic in range(CIC):
                    mm(qcol[:, j:j + 1], yv[:, cic, j * 128:(j + 1) * 128],
                       wbar[:, cic:cic + 1], cic == 0, cic == CIC - 1)
            pm = rtmp.tile([128, MC], F32, tag="pm", name="pm")
            nc.vector.tensor_mul(pm[:], rcol[:], qcol[:])
            nc.vector.tensor_add(pooledT[:, :, s], pm[:], rs_sb[:])

        # Software pipeline: sample s's exp-dependent stages (Z, U, fixup)
        # are deferred into sample s+1's sections, where every exp of sample
        # s has long finished - the PE never waits on Scalar.  The
        # attention-apply is split: cic0 into the x-section (which has
        # Scalar slack), cic1 into the exp-heavy y-section.
        prev = None
        prev_yv = None
        prev_p1 = None
        prev_rcol = None
        for s in range(BPC):
            x8, y8, rs_sb = in_tiles[s]
            # prefetch sample s+1's inputs a full section ahead: issued at
            # the end of the body they land ~1.7us into s+1's x-section and
            # stall its theta projection every sample
            if s + 1 < BPC:
                in_tiles.append(load_inputs(s + 1))
            streams = (("x", x8), ("y", y8))
            pj = {}
            gT = {}
            E = {}
            S = attp.tile([128, MC, N], FP8W, tag="S", name="S")
            for st, src in streams:
                for pr in ("t", "p"):
                    dst = proj.tile([128, CIC, N], FP8, tag=f"pj_{pr}{st}",
                                    name=f"pj_{pr}{st}")
                    pj[pr + st] = dst
                    proj_tile(w_sb[f"wt_{pr}{st}"], src,
                              tb_sb.get(f"b_{pr}{st}"), dst, pr == "t")
                # deferred stages of the previous sample
                if prev is not None:
                    if st == "x":
                        prev_p1 = emit_Z(prev)
                        prev_yv = yvp.tile([128, CIC, N], BF16, tag="yv",
                                           name="yv")
                        emit_U_cic(prev, prev_yv, 0, False)
                    else:
                        emit_U_cic(prev, prev_yv, 1, True)

                # logits interleaved 1:1 with g tiles: the Scalar EXP stream
                # trails the logits tiles; the g tiles in between drain via
                # DVE, so the psum rotation never stalls the PE on a
                # pending exp
                wg = w_sb[f"wt_g{st}"]
                gdst = proj.tile([128, MC, CI], FP8, tag=f"gT{st}",
                                 name=f"gT{st}")
                gT[st] = gdst
                has_b = st in gb_sb
                edst = attp.tile([128, MC, N], FP8W, tag=f"E{st}", name=f"E{st}")
                E[st] = edst
                gps = None
                for mc_ in range(MC):
                    # g psums packed three mc-chunks per wide tile
                    if mc_ % 3 == 0:
                        gps = psM.tile([128, 3, CI], F32, tag="M", name="gps")
                    gh = gps[:, mc_ % 3]
                    for kp in range(KC // 2):
                        mmdr(gh,
                             src[:, 2 * kp:2 * kp + 2, mc_ * 128:(mc_ + 1) * 128],
                             wg[:, 2 * kp:2 * kp + 2, :],
                             kp == 0, (kp == KC // 2 - 1) and not has_b)
                    if has_b:
                        mm(gh, ones_row[:], gb_sb[st][:], False, True,
                           skip_group_check=True)
                    if mc_ % 3 == 2:
                        nc.vector.tensor_copy(gdst[:, mc_ - 2:mc_ + 1, :],
                                              gps[:])
                    lt = psM.tile([128, N], F32, tag="M", name="logits")
                    for o, f in NH:
                        mmdr(lt[:, o:o + f],
                             pj["p" + st][:, :, mc_ * 128:(mc_ + 1) * 128],
                             pj["t" + st][:, :, o:o + f], True, True)
                    nc.scalar.activation(edst[:, mc_, :], lt[:], AF.Exp,
                                         bias=expb[:])
                    if st == "y" and s == BPC - 1 and mc_ >= MC - 2:
                        # last sample: final pair split per-mc so the tail's
                        # attention-apply chain starts half a mul earlier
                        nc.vector.tensor_mul(S[:, mc_:mc_ + 1, :],
                                             E["x"][:, mc_:mc_ + 1, :],
                                             E["y"][:, mc_:mc_ + 1, :])
                        if mc_ == MC - 2:
                            # its own Zx matmuls fill the y-loop's exp-paced
                            # stall window (E-x complete since the x-section)
                            # and the zx drain runs here, between exps, so
                            # the tail's psum ring never waits on it behind
                            # the trailing exp backlog
                            last_zx = emit_Z_key(E["x"])
                            last_zx_sb = rtmp.tile([1, N], F32, tag="zx_sb",
                                                   name="zx_sb")
                            nc.scalar.copy(last_zx_sb[:], last_zx[:])
                    elif st == "y" and mc_ % 2 == 1:
                        # fused map product per chunk-pair (adjacent free dim)
                        nc.vector.tensor_mul(S[:, mc_ - 1:mc_ + 1, :],
                                             E["x"][:, mc_ - 1:mc_ + 1, :],
                                             E["y"][:, mc_ - 1:mc_ + 1, :])
                    elif st == "x" and mc_ == MC - 1 and prev is not None:
                        prev_rcol = emit_T(prev_p1)
                if st == "y" and prev is not None:
                    emit_Q(prev, prev_yv, prev_rcol)

            prev = (s, E, S, gT, rs_sb)

        # drain the last sample
        p1 = emit_Z(prev, last_zx_sb)
        yv = yvp.tile([128, CIC, N], BF16, tag="yv", name="yv")
        # tail: with the zx drain moved into the y-loop, the Scalar queue
        # still holds the trailing exps, so bounce Ux through DVE
        emit_U_cic(prev, yv, 0, False)
        rcol = emit_T(p1)
        # cic1 attention-apply with a latency-trimmed tail: the qraw-cic0
        # matvecs run while cic1 is still in flight, and qraw-cic1 follows
        # the ux/yv bounce as it lands
        _, _, S_l, gT_l, rs_l = prev
        ptu = {}
        for st in ("x", "y"):
            ptu[st] = psM.tile([128, N], F32, tag="M", name=f"U{st}l")
            for o, f in NH:
                for jp in range(MC // 2):
                    mmdr(ptu[st][:, o:o + f],
                         gT_l[st][:, 2 * jp:2 * jp + 2, 128:256],
                         S_l[:, 2 * jp:2 * jp + 2, o:o + f],
                         jp == 0, jp == MC // 2 - 1)
        qcol = psM.tile([128, MC], F32, tag="M", name="qcol")
        for j in range(MC):
            mm(qcol[:, j:j + 1], yv[:, 0, j * 128:(j + 1) * 128],
               wbar[:, 0:1], True, False)
        ux_sb = yvp.tile([128, N], BF16, tag="ux_sb", name="ux_sb")
        nc.scalar.copy(ux_sb[:], ptu["x"][:])
        nc.vector.tensor_mul(yv[:, 1, :], ux_sb[:], ptu["y"][:])
        for j in range(MC):
            mm(qcol[:, j:j + 1], yv[:, 1, j * 128:(j + 1) * 128],
               wbar[:, 1:2], False, j == MC - 1)
        pm = rtmp.tile([128, MC], F32, tag="pm", name="pm")
        nc.vector.tensor_mul(pm[:], rcol[:], qcol[:])
        nc.vector.tensor_add(pooledT[:, :, BPC - 1], pm[:], rs_l[:])

        # ---- head ----
        pt = psM.tile([BPC, HOUT], F32, tag="M", name="head_ps")
        for j in range(MC):
            mm(pt[:], pooledT[:, j, :], hwT[:, j, :],
               j == 0, (j == MC - 1) and not has_hb)
        if has_hb:
            mm(pt[:], ones_row[:, :BPC], hb[:], False, True)
        out_sb = rows.tile([BPC, HOUT], F32, tag="out_sb", name="out_sb")
        nc.scalar.copy(out_sb[:], pt[:])
        # issue the output DMA from the Scalar queue that produced out_sb:
        # FIFO order replaces the cross-engine semaphore hop to Sync
        nc.scalar.dma_start(d_out[:], out_sb[:])

    nc.compile()
    return nc


def _prepare(inputs):
    f = lambda k: np.ascontiguousarray(np.asarray(inputs[k], dtype=np.float32))
    bf = lambda a: np.ascontiguousarray(np.asarray(a, dtype=ml_dtypes.bfloat16))
    sar, opt = f("sar"), f("opt")
    ga = float(np.asarray(inputs["gamma_att"]).reshape(-1)[0])
    go = float(np.asarray(inputs["gamma_opt"]).reshape(-1)[0])
    gs = float(np.asarray(inputs["gamma_sar"]).reshape(-1)[0])
    W_w, W_b = f("W_w"), f("W_b")
    head_w, head_b = f("head_w"), f("head_b")

    wbar = (ga / C) * W_w.sum(axis=0)  # (CI,)
    bbar = (ga / C) * float(W_b.sum())
    # fold the pooled-constant through the head: out += bbar * head_w.sum(1)
    hb_eff = head_b + bbar * head_w.sum(axis=1)  # (HOUT,)

    tb = np.stack([f("theta_sar_b"), f("phi_sar_b"),
                   f("theta_opt_b"), f("phi_opt_b")])
    has_tb = bool(np.any(tb))
    gb_x, gb_y = f("g_sar_b"), f("g_opt_b")
    has_gb_x = bool(np.any(gb_x))
    has_gb_y = bool(np.any(gb_y))
    has_hb = bool(np.any(hb_eff))

    key = (has_tb, has_gb_x, has_gb_y, has_hb)
    if key not in _cached:
        _cached[key] = _build(*key)
    nc = _cached[key]

    # pack inputs: (B, C, N) -> per-core (BPC, 128, KC*N) partition-major fp8
    def pack_in(a):
        a = a.reshape(B, KC, 128, N).transpose(0, 2, 1, 3).reshape(B, 128, KC * N)
        return np.ascontiguousarray(a).astype(ml_dtypes.float8_e4m3fn)

    sar_p, opt_p = pack_in(sar), pack_in(opt)

    # exact residual + channel-mean pool term, per-sample column layout
    rs = (go / C) * opt.sum(axis=1) + (gs / C) * sar.sum(axis=1)  # (B, N)
    rs = np.ascontiguousarray(
        rs.reshape(B, MC, 128).transpose(0, 2, 1)).astype(np.float32)

    common = {
        "wt_tx": _pack(f("theta_sar_w").T),
        "wt_px": _pack(f("phi_sar_w").T),
        "wt_ty": _pack(f("theta_opt_w").T),
        "wt_py": _pack(f("phi_opt_w").T),
        "wt_gx": _pack(f("g_sar_w").T),
        "wt_gy": _pack(f("g_opt_w").T),
        "hwT": np.ascontiguousarray(
            head_w.T.reshape(MC, 128, HOUT).transpose(1, 0, 2)
            .reshape(128, MC * HOUT)).astype(ml_dtypes.bfloat16),
        "wbar": bf(wbar),
    }
    if has_tb:
        common["tb"] = np.ascontiguousarray(tb)
    if has_gb_x or has_gb_y or has_hb:
        common["ones_row"] = np.ones((1, 128), ml_dtypes.bfloat16)
    if has_gb_x:
        common["gb_x"] = bf(gb_x.reshape(1, CI))
    if has_gb_y:
        common["gb_y"] = bf(gb_y.reshape(1, CI))
    if has_hb:
        common["hb"] = bf(hb_eff.reshape(1, HOUT))

    in_maps = []
    for c in range(NCORES):
        m = dict(common)
        m["sar8"] = np.ascontiguousarray(sar_p[c * BPC:(c + 1) * BPC])
        m["opt8"] = np.ascontiguousarray(opt_p[c * BPC:(c + 1) * BPC])
        m["rs"] = np.ascontiguousarray(rs[c * BPC:(c + 1) * BPC])
        in_maps.append(m)
    return nc, in_maps


def kernel(**inputs):
    nc, in_maps = _prepare(inputs)
    res = run_bass_kernel_spmd(nc, in_maps, core_ids=list(range(NCORES)))
    return np.concatenate([res.results[c]["out"] for c in range(NCORES)], axis=0)


if __name__ == "__main__":
    rng = np.random.default_rng(0)
    ins = {
        "sar": rng.standard_normal((B, C, N), dtype=np.float32),
        "opt": rng.standard_normal((B, C, N), dtype=np.float32),
    }
    for nm in ("g_sar", "g_opt", "theta_sar", "theta_opt", "phi_sar", "phi_opt"):
        ins[nm + "_w"] = 0.02 * rng.standard_normal((CI, C), dtype=np.float32)
        ins[nm + "_b"] = np.zeros((CI,), np.float32)
    ins["W_w"] = 0.02 * rng.standard_normal((C, CI), dtype=np.float32)
    ins["W_b"] = np.zeros((C,), np.float32)
    ins["head_w"] = 0.02 * rng.standard_normal((HOUT, N), dtype=np.float32)
    ins["head_b"] = np.zeros((HOUT,), np.float32)
    ins["gamma_sar"] = np.asarray([0.3], np.float32)
    ins["gamma_opt"] = np.asarray([1.0], np.float32)
    ins["gamma_att"] = np.asarray([1.0], np.float32)
    out = kernel(**ins)
    print(out.shape, out.dtype, np.abs(out).mean())
